# revision 40
# baseline (speedup 1.0000x reference)
"""Trainium2 Bass kernel for nn_DifferentiableAggregation (segment_reduce).

Computes, for batch of 8192 segments over 8388608 sub-images:
    s0[g]  = sum over i with idx_i == g of sub_logits[i, 0]
    s12[g] = sum over i with idx_i == g of (sub_logits[i, 1] + sub_logits[i, 2])
    out[g] = [log(sigmoid(10*(1-s12[g])) + 1e-10),
              log(sigmoid(10*(5-s0[g]))  + 1e-10)]

Strategy: shard the sub-image axis across 8 NeuronCores. Each core does a
local segment-sum via one-hot matmuls accumulating in PSUM (index split as
g = r*64 + q with r = idx>>6 on the 128 PSUM partitions and q = idx&63 in
the free dim), then an AllReduce of the [128, 128] partial and the
sigmoid/log epilogue.

The r one-hot (the matmul lhsT) is shipped prebuilt by the host as fp8e4
bytes — a pure per-element index recoding (128 B/elem, ~134 MB/core of
overlapped DMA) consumed directly by the PE as mixed fp8xfp16 matmuls
(HW-verified exact and ~63 ns per LDWEIGHTS+MM pair). The q one-hot and
the value placement run on the DVE in fp16 at the 2x_1P packed rate.
The 2x mode requires every operand's innermost AP dim to be step-1 with
>= 2 elements, which a plain per-element broadcast (innermost stride 0)
violates. Trick: the host ships each per-element scalar (w0, w12, q)
DUPLICATED x2, so broadcasts become [p, j(:6), k(:0), d(:1)] — the
innermost dim is a real 4B-aligned step-1 pair and the stride-0 repeat
moves to a middle dim, which the packed mode tolerates (HW-verified).
One-hot tensors stay k-inner so the matmuls consume dense [p, 128]
slices.
"""

import sys

sys.path.insert(0, "/opt/trn_rl_repo")

import numpy as np

from concourse import bass, bacc, mybir, tile
from concourse.bass_utils import run_bass_kernel_spmd

N_CORES = 8
TOTAL = 8388608
SHARD = TOTAL // N_CORES  # 1048576
BATCH = 8192
P = 128
F32 = mybir.dt.float32
F16 = mybir.dt.float16
U8 = mybir.dt.uint8
F8 = mybir.dt.float8e4
ONE_F8 = 0x38  # fp8e4m3 bit pattern of 1.0

K_SHARP = 10.0
EPS = 1e-10


def build_nc(to_count, ti):
    """Build + compile the SPMD bass program. Per core handles
    to_count * 128 * ti elements; 8 duplicated fp16 words per element."""
    shard = to_count * P * ti
    assert shard == SHARD
    nc = bacc.Bacc(
        "TRN2",
        debug=False,
        target_bir_lowering=False,
        num_devices=N_CORES,
    )
    d_in = nc.dram_tensor("data", [shard * 6], F16, kind="ExternalInput")
    b3_in = nc.dram_tensor("b3oh", [shard * 128], U8, kind="ExternalInput")
    oh_in = nc.dram_tensor("ohq16", [(shard // 4) * 64], F16,
                           kind="ExternalInput")
    io64_in = nc.dram_tensor("iota64", [P, 64], F16, kind="ExternalInput")
    out_logits = nc.dram_tensor("logits", [2, BATCH], F32, kind="ExternalOutput")

    with tile.TileContext(nc) as tc:
        _kernel_body(tc, to_count, ti, d_in, b3_in, oh_in, io64_in, out_logits)
    nc.compile()
    return nc


def _kernel_body(tc, to_count, ti, d_in, b3_in, oh_in, io64_in, out_logits):
    nc = tc.nc
    add = mybir.AluOpType.add
    is_equal = mybir.AluOpType.is_equal
    mult = mybir.AluOpType.mult
    AF = mybir.ActivationFunctionType

    dv = d_in.ap().rearrange("(o p f) -> o p f", p=P, f=ti * 6)
    b3v = b3_in.ap().rearrange("(o p f) -> o p f", p=P, f=ti * 128)
    ohv = oh_in.ap().rearrange("(o p f) -> o p f", p=P, f=ti * 64)

    S = min(128, ti)  # elements per partition per micro-block
    assert ti % S == 0
    nb = ti // S

    with (
        tc.tile_pool(name="const", bufs=1) as cpool,
        tc.tile_pool(name="data", bufs=4) as dpool,
        tc.tile_pool(name="onehot", bufs=3) as bpool,
        tc.tile_pool(name="vqp", bufs=2) as vpool,
        tc.tile_pool(name="mid", bufs=4) as mpool,
        tc.tile_pool(name="psum", bufs=1, space="PSUM") as ppool,
        tc.tile_pool(name="epi", bufs=2) as epool,
        tc.tile_pool(name="dram", bufs=1, space="DRAM") as drampool,
    ):
        io64 = cpool.tile([P, 64], F16)
        nc.sync.dma_start(io64[:], io64_in.ap())
        # [p, (j:0), k(:1)] — broadcast over j on a middle dim
        io64b = io64[:].rearrange("p (o k) -> p o k", o=1).to_broadcast([P, S, 64])
        io64b32 = io64[:].rearrange("p (o k) -> p o k", o=1).to_broadcast(
            [P, 32, 64]
        )

        acc_e = ppool.tile([P, 128], F32, tag="acc_e")
        acc_o = ppool.tile([P, 128], F32, tag="acc_o")

        for to in range(to_count):
            dt = dpool.tile([P, ti * 6], F16, tag="dt")
            nc.sync.dma_start(dt[:], dv[to])
            # per element u-layout: [w0, w0, w12, w12, q, q]
            du = dt[:].rearrange("p (j u) -> p j u", u=6)

            # r one-hot arrives prebuilt from the host as fp8 bytes
            # (pure index recoding; 128 B/elem, overlapped DMA)
            B_all = bpool.tile([P, ti * 128], U8, tag="B")
            nc.sync.dma_start(B_all[:], b3v[to])
            B3 = B_all[:].bitcast(F8).rearrange("p (j k) -> p j k", k=128)

            half = to_count // 2
            # sub-block the first & last to-blocks: the first so compute
            # starts before the full 2 MB B3 slab lands, the last so the
            # final (serial) matmul chain is short
            if to in (0, to_count - 1):
                chunks = [(c * 32, 32) for c in range(ti // 32)]
            else:
                chunks = [(0, ti)]
            for off0, Sc in chunks:
                dsl = du[:, off0:off0 + Sc]
                iob = io64b if Sc == ti else io64b32

                def dup_pair(off, k_half, dsl=dsl, Sc=Sc):
                    # [p, j(:6), k(:0), d(:1)] — innermost step-1 pair
                    return (
                        dsl[:, :, off:off + 2]
                        .rearrange("p j (o d) -> p j o d", o=1)
                        .to_broadcast([P, Sc, k_half, 2])
                    )

                # q one-hot: OHQ[p, j, k] = (q[p, j] == k)
                # (full-size tiles reused for the 32-wide end chunks)
                # Every 4th block's one-hot ships prebuilt from the host
                # (fp16, another pure index recoding) — skips the IS_EQ and
                # rebalances DVE vs the underused DMA engines, while total
                # streamed input stays below the SBUF-contention cliff.
                OHQ_full = mpool.tile([P, ti * 64], F16, tag="OHQ")
                OHQ_all = OHQ_full[:, :Sc * 64]
                if to % 4 == 2:
                    assert Sc == ti
                    nc.sync.dma_start(OHQ_full[:], ohv[to // 4])
                else:
                    OHQd = OHQ_all.rearrange("p (j k d) -> p j k d", k=32, d=2)
                    io64d = iob.rearrange("p j (k d) -> p j k d", d=2)
                    nc.vector.tensor_tensor(
                        OHQd, dup_pair(4, 32), io64d, is_equal
                    )

                # VQ[p, j, c, k] = w_c[p, j] * OHQ[p, j, k]  (c-slice per TT)
                VQ_full = vpool.tile([P, ti * 2 * 64], F16, tag="VQ")
                VQ_all = VQ_full[:, :Sc * 128]
                VQ4 = VQ_all.rearrange("p (j c k) -> p j c k", c=2, k=64)
                for c, off in ((0, 0), (1, 2)):
                    vqc = VQ4[:, :, c].rearrange("p j (k d) -> p j k d", d=2)
                    ohqd = OHQ_all.rearrange(
                        "p (j k d) -> p j k d", k=32, d=2
                    )
                    nc.vector.tensor_tensor(vqc, dup_pair(off, 32), ohqd, mult)

                VQn = VQ_all.rearrange("p (j n) -> p j n", n=128)
                pacc = acc_e if to < half else acc_o
                for j in range(Sc):
                    first = to in (0, half) and off0 == 0 and j == 0
                    last = (
                        to in (half - 1, to_count - 1) and off0 + j == ti - 1
                    )
                    nc.tensor.matmul(
                        pacc[:],
                        lhsT=B3[:, off0 + j, :],
                        rhs=VQn[:, j, :],
                        start=first,
                        stop=last,
                    )

            # Split AllReduce: acc_e's accumulation group closes at the
            # halfway block, so its collective (which eats the
            # cross-core barrier skew and most of the ring latency)
            # fully overlaps the second half's compute. Only acc_o's
            # collective sits on the tail, with cores synced.
            if to == half - 1:
                groups = [list(range(N_CORES))]
                s_e = epool.tile([P, 128], F16)
                nc.vector.tensor_copy(s_e[:], acc_e[:])
                din_e = drampool.tile([P, 128], F16)
                dout_e = drampool.tile([P, 128], F16)
                nc.sync.dma_start(din_e[:], s_e[:])
                nc.gpsimd.collective_compute(
                    "AllReduce", add, replica_groups=groups,
                    ins=[din_e.opt()], outs=[dout_e.opt()],
                )

        s_o = epool.tile([P, 128], F16)
        nc.vector.tensor_copy(s_o[:], acc_o[:])
        din_o = drampool.tile([P, 128], F16)
        dout_o = drampool.tile([P, 128], F16)
        nc.sync.dma_start(din_o[:], s_o[:])
        nc.gpsimd.collective_compute(
            "AllReduce", add, replica_groups=groups,
            ins=[din_o.opt()], outs=[dout_o.opt()],
        )

        sf_e = epool.tile([P, 128], F16)
        nc.sync.dma_start(sf_e[:], dout_e[:])
        sf_o = epool.tile([P, 128], F16)
        nc.sync.dma_start(sf_o[:], dout_o[:])
        sf = epool.tile([P, 128], F32)
        nc.vector.tensor_tensor(sf[:], sf_e[:], sf_o[:], add)

        # Epilogue: out_c = log(sigmoid(z) + eps), z = -10*s + bias_c.
        # sigmoid computed exactly as 1/(1 + exp(-z)) (ACT exp table +
        # accurate DVE reciprocal); -z clamped at 88 to avoid exp
        # overflow (beyond that sigmoid+eps == eps in fp32 anyway).
        # exp and ln share one ACT table set, so no table swapping.
        beps = epool.tile([P, 1], F32)
        nc.vector.memset(beps[:], EPS)

        def logsig(out_ap, s_ap, zbias):
            mz = epool.tile([P, 64], F32, tag="mz")
            nc.vector.tensor_scalar(mz[:], s_ap, K_SHARP, -zbias,
                                    mybir.AluOpType.mult, mybir.AluOpType.add)
            nc.vector.tensor_scalar(mz[:], mz[:], 88.0, None,
                                    mybir.AluOpType.min)
            w = epool.tile([P, 64], F32, tag="w")
            nc.scalar.activation(w[:], mz[:], AF.Exp, bias=0.0, scale=1.0)
            nc.vector.tensor_scalar(w[:], w[:], 1.0, None,
                                    mybir.AluOpType.add)
            r = epool.tile([P, 64], F32, tag="r")
            nc.vector.reciprocal(r[:], w[:])
            nc.scalar.activation(out_ap, r[:], AF.Ln, bias=beps[:], scale=1.0)

        o1 = epool.tile([P, 64], F32)
        logsig(o1[:], sf[:, 64:128], K_SHARP)
        o0 = epool.tile([P, 64], F32)
        logsig(o0[:], sf[:, 0:64], 5.0 * K_SHARP)

        ol = out_logits.ap().rearrange("w (p t) -> w p t", p=P, t=BATCH // P)
        nc.sync.dma_start(ol[0], o1[:])
        nc.sync.dma_start(ol[1], o0[:])


_NC_CACHE = {}


def _get_nc(to_count, ti):
    key = (to_count, ti)
    if key not in _NC_CACHE:
        _NC_CACHE[key] = build_nc(to_count, ti)
    return _NC_CACHE[key]


def make_in_maps(sub_logits, original_indices, to_count, ti):
    idx = np.asarray(original_indices).astype(np.int32)
    v = np.asarray(sub_logits, dtype=np.float32)
    w0 = v[:, 0].astype(np.float16)
    w12 = (v[:, 1] + v[:, 2]).astype(np.float16)
    q_f = (idx & 63).astype(np.float16)
    # duplicated-pair element layout: [w0, w0, w12, w12, q, q]
    packed = np.empty((TOTAL, 6), dtype=np.float16)
    packed[:, 0] = w0
    packed[:, 1] = w0
    packed[:, 2] = w12
    packed[:, 3] = w12
    packed[:, 4] = q_f
    packed[:, 5] = q_f
    packed = packed.reshape(N_CORES, SHARD * 6)

    # r one-hot prebuilt as fp8e4m3 bytes (1.0 = 0x38)
    b3 = np.zeros((TOTAL, 128), dtype=np.uint8)
    b3[np.arange(TOTAL), idx >> 6] = ONE_F8
    b3 = b3.reshape(N_CORES, SHARD * 128)

    # q one-hot prebuilt in fp16 for every 4th block (to % 4 == 2)
    nblk = SHARD // (P * ti)
    q4 = (idx & 63).reshape(N_CORES, nblk, P, ti)[:, 2::4]
    ohq = np.zeros(q4.shape + (64,), dtype=np.float16)
    np.put_along_axis(
        ohq, q4[..., None].astype(np.int64),
        np.ones_like(q4, dtype=np.float16)[..., None], axis=4,
    )
    ohq = ohq.reshape(N_CORES, (SHARD // 4) * 64)

    io64 = np.ascontiguousarray(
        np.broadcast_to(np.arange(64, dtype=np.float16), (P, 64))
    )
    return [
        {"data": packed[c], "b3oh": b3[c], "ohq16": ohq[c], "iota64": io64}
        for c in range(N_CORES)
    ]


def kernel(sub_logits, original_indices, batch_size=None, _trace=False):
    to_count, ti = 64, 128
    nc = _get_nc(to_count, ti)
    in_maps = make_in_maps(sub_logits, original_indices, to_count, ti)
    res = run_bass_kernel_spmd(
        nc, in_maps, core_ids=list(range(N_CORES)), trace=_trace
    )
    logits = res.results[0]["logits"]
    out = np.stack([logits[0], logits[1]], axis=1).astype(np.float32)
    if _trace:
        kernel._last_results = res
    return out


# revision 42
# speedup vs baseline: 1.0354x; 1.0354x over previous
"""Trainium2 Bass kernel for nn_DifferentiableAggregation (segment_reduce).

Computes, for batch of 8192 segments over 8388608 sub-images:
    s0[g]  = sum over i with idx_i == g of sub_logits[i, 0]
    s12[g] = sum over i with idx_i == g of (sub_logits[i, 1] + sub_logits[i, 2])
    out[g] = [log(sigmoid(10*(1-s12[g])) + 1e-10),
              log(sigmoid(10*(5-s0[g]))  + 1e-10)]

Strategy: shard the sub-image axis across 8 NeuronCores. Each core does a
local segment-sum via one-hot matmuls accumulating in PSUM (index split as
g = r*64 + q with r = idx>>6 on the 128 PSUM partitions and q = idx&63 in
the free dim), then an AllReduce of the [128, 128] partial and the
sigmoid/log epilogue.

The r one-hot (the matmul lhsT) is shipped prebuilt by the host as fp8e4
bytes — a pure per-element index recoding (128 B/elem, ~134 MB/core of
overlapped DMA) consumed directly by the PE as mixed fp8xfp16 matmuls
(HW-verified exact and ~63 ns per LDWEIGHTS+MM pair). The q one-hot and
the value placement run on the DVE in fp16 at the 2x_1P packed rate.
The 2x mode requires every operand's innermost AP dim to be step-1 with
>= 2 elements, which a plain per-element broadcast (innermost stride 0)
violates. Trick: the host ships each per-element scalar (w0, w12, q)
DUPLICATED x2, so broadcasts become [p, j(:6), k(:0), d(:1)] — the
innermost dim is a real 4B-aligned step-1 pair and the stride-0 repeat
moves to a middle dim, which the packed mode tolerates (HW-verified).
One-hot tensors stay k-inner so the matmuls consume dense [p, 128]
slices.
"""

import sys

sys.path.insert(0, "/opt/trn_rl_repo")

import numpy as np

from concourse import bass, bacc, mybir, tile
from concourse.bass_utils import run_bass_kernel_spmd

N_CORES = 8
TOTAL = 8388608
SHARD = TOTAL // N_CORES  # 1048576
BATCH = 8192
P = 128
F32 = mybir.dt.float32
F16 = mybir.dt.float16
U8 = mybir.dt.uint8
F8 = mybir.dt.float8e4
ONE_F8 = 0x38  # fp8e4m3 bit pattern of 1.0

K_SHARP = 10.0
EPS = 1e-10


def build_nc(to_count, ti):
    """Build + compile the SPMD bass program. Per core handles
    to_count * 128 * ti elements; 8 duplicated fp16 words per element."""
    shard = to_count * P * ti
    assert shard == SHARD
    nc = bacc.Bacc(
        "TRN2",
        debug=False,
        target_bir_lowering=False,
        num_devices=N_CORES,
    )
    d_in = nc.dram_tensor("data", [shard * 6], F16, kind="ExternalInput")
    b3_in = nc.dram_tensor("b3oh", [shard * 128], U8, kind="ExternalInput")
    oh_in = nc.dram_tensor("ohq16", [(shard // 4) * 64], F16,
                           kind="ExternalInput")
    io64_in = nc.dram_tensor("iota64", [P, 64], F16, kind="ExternalInput")
    out_logits = nc.dram_tensor("logits", [2, BATCH], F32, kind="ExternalOutput")

    with tile.TileContext(nc) as tc:
        _kernel_body(tc, to_count, ti, d_in, b3_in, oh_in, io64_in, out_logits)
    nc.compile()
    return nc


def _kernel_body(tc, to_count, ti, d_in, b3_in, oh_in, io64_in, out_logits):
    nc = tc.nc
    add = mybir.AluOpType.add
    is_equal = mybir.AluOpType.is_equal
    mult = mybir.AluOpType.mult
    AF = mybir.ActivationFunctionType

    dv = d_in.ap().rearrange("(o p f) -> o p f", p=P, f=ti * 6)
    b3v = b3_in.ap().rearrange("(o p f) -> o p f", p=P, f=ti * 128)
    ohv = oh_in.ap().rearrange("(o p f) -> o p f", p=P, f=ti * 64)

    S = min(128, ti)  # elements per partition per micro-block
    assert ti % S == 0
    nb = ti // S

    with (
        tc.tile_pool(name="const", bufs=1) as cpool,
        tc.tile_pool(name="data", bufs=4) as dpool,
        tc.tile_pool(name="onehot", bufs=3) as bpool,
        tc.tile_pool(name="vqp", bufs=2) as vpool,
        tc.tile_pool(name="mid", bufs=3) as mpool,
        tc.tile_pool(name="psum", bufs=1, space="PSUM") as ppool,
        tc.tile_pool(name="epi", bufs=2) as epool,
        tc.tile_pool(name="dram", bufs=1, space="DRAM") as drampool,
    ):
        io64 = cpool.tile([P, 64], F16)
        nc.sync.dma_start(io64[:], io64_in.ap())
        # [p, (j:0), k(:1)] — broadcast over j on a middle dim
        io64b = io64[:].rearrange("p (o k) -> p o k", o=1).to_broadcast([P, S, 64])
        io64b32 = io64[:].rearrange("p (o k) -> p o k", o=1).to_broadcast(
            [P, 32, 64]
        )

        acc_e = ppool.tile([P, 128], F32, tag="acc_e")
        acc_o = ppool.tile([P, 128], F32, tag="acc_o")

        for to in range(to_count):
            dt = dpool.tile([P, ti * 6], F16, tag="dt")
            nc.sync.dma_start(dt[:], dv[to])
            # per element u-layout: [w0, w0, w12, w12, q, q]
            du = dt[:].rearrange("p (j u) -> p j u", u=6)

            # r one-hot arrives prebuilt from the host as fp8 bytes
            # (pure index recoding; 128 B/elem, overlapped DMA)
            B_all = bpool.tile([P, ti * 128], U8, tag="B")
            nc.sync.dma_start(B_all[:], b3v[to])
            B3 = B_all[:].bitcast(F8).rearrange("p (j k) -> p j k", k=128)

            half = (to_count * 7) // 8
            # sub-block the first & last to-blocks: the first so compute
            # starts before the full 2 MB B3 slab lands, the last so the
            # final (serial) matmul chain is short
            if to in (0, to_count - 1):
                chunks = [(c * 32, 32) for c in range(ti // 32)]
            else:
                chunks = [(0, ti)]
            for off0, Sc in chunks:
                dsl = du[:, off0:off0 + Sc]
                iob = io64b if Sc == ti else io64b32

                def dup_pair(off, k_half, dsl=dsl, Sc=Sc):
                    # [p, j(:6), k(:0), d(:1)] — innermost step-1 pair
                    return (
                        dsl[:, :, off:off + 2]
                        .rearrange("p j (o d) -> p j o d", o=1)
                        .to_broadcast([P, Sc, k_half, 2])
                    )

                # q one-hot: OHQ[p, j, k] = (q[p, j] == k)
                # (full-size tiles reused for the 32-wide end chunks)
                # Every 4th block's one-hot ships prebuilt from the host
                # (fp16, another pure index recoding) — skips the IS_EQ and
                # rebalances DVE vs the underused DMA engines, while total
                # streamed input stays below the SBUF-contention cliff.
                OHQ_full = mpool.tile([P, ti * 64], F16, tag="OHQ")
                OHQ_all = OHQ_full[:, :Sc * 64]
                if to % 4 == 2:
                    assert Sc == ti
                    nc.sync.dma_start(OHQ_full[:], ohv[to // 4])
                else:
                    OHQd = OHQ_all.rearrange("p (j k d) -> p j k d", k=32, d=2)
                    io64d = iob.rearrange("p j (k d) -> p j k d", d=2)
                    nc.vector.tensor_tensor(
                        OHQd, dup_pair(4, 32), io64d, is_equal
                    )

                # VQ[p, j, c, k] = w_c[p, j] * OHQ[p, j, k]  (c-slice per TT)
                VQ_full = vpool.tile([P, ti * 2 * 64], F16, tag="VQ")
                VQ_all = VQ_full[:, :Sc * 128]
                VQ4 = VQ_all.rearrange("p (j c k) -> p j c k", c=2, k=64)
                for c, off in ((0, 0), (1, 2)):
                    vqc = VQ4[:, :, c].rearrange("p j (k d) -> p j k d", d=2)
                    ohqd = OHQ_all.rearrange(
                        "p (j k d) -> p j k d", k=32, d=2
                    )
                    nc.vector.tensor_tensor(vqc, dup_pair(off, 32), ohqd, mult)

                VQn = VQ_all.rearrange("p (j n) -> p j n", n=128)
                pacc = acc_e if to < half else acc_o
                for j in range(Sc):
                    first = to in (0, half) and off0 == 0 and j == 0
                    last = (
                        to in (half - 1, to_count - 1) and off0 + j == ti - 1
                    )
                    nc.tensor.matmul(
                        pacc[:],
                        lhsT=B3[:, off0 + j, :],
                        rhs=VQn[:, j, :],
                        start=first,
                        stop=last,
                    )

            # Split AllReduce: acc_e's accumulation group closes at the
            # halfway block, so its collective (which eats the
            # cross-core barrier skew and most of the ring latency)
            # fully overlaps the second half's compute. Only acc_o's
            # collective sits on the tail, with cores synced.
            if to == half - 1:
                groups = [list(range(N_CORES))]
                s_e = epool.tile([P, 128], F32)
                nc.vector.tensor_copy(s_e[:], acc_e[:])
                din_e = drampool.tile([P, 128], F32)
                dout_e = drampool.tile([P, 128], F32)
                nc.sync.dma_start(din_e[:], s_e[:])
                nc.gpsimd.collective_compute(
                    "AllReduce", add, replica_groups=groups,
                    ins=[din_e.opt()], outs=[dout_e.opt()],
                )

        s_o = epool.tile([P, 128], F32)
        nc.vector.tensor_copy(s_o[:], acc_o[:])
        din_o = drampool.tile([P, 128], F32)
        dout_o = drampool.tile([P, 128], F32)
        nc.sync.dma_start(din_o[:], s_o[:])
        nc.gpsimd.collective_compute(
            "AllReduce", add, replica_groups=groups,
            ins=[din_o.opt()], outs=[dout_o.opt()],
        )

        sf_e = epool.tile([P, 128], F32)
        nc.sync.dma_start(sf_e[:], dout_e[:])
        sf_o = epool.tile([P, 128], F32)
        nc.sync.dma_start(sf_o[:], dout_o[:])
        sf = epool.tile([P, 128], F32)
        nc.vector.tensor_tensor(sf[:], sf_e[:], sf_o[:], add)

        # Epilogue: out_c = log(sigmoid(z) + eps), z = -10*s + bias_c.
        # sigmoid computed exactly as 1/(1 + exp(-z)) (ACT exp table +
        # accurate DVE reciprocal); -z clamped at 88 to avoid exp
        # overflow (beyond that sigmoid+eps == eps in fp32 anyway).
        # exp and ln share one ACT table set, so no table swapping.
        beps = epool.tile([P, 1], F32)
        nc.vector.memset(beps[:], EPS)

        def logsig(out_ap, s_ap, zbias):
            mz = epool.tile([P, 64], F32, tag="mz")
            nc.vector.tensor_scalar(mz[:], s_ap, K_SHARP, -zbias,
                                    mybir.AluOpType.mult, mybir.AluOpType.add)
            nc.vector.tensor_scalar(mz[:], mz[:], 88.0, None,
                                    mybir.AluOpType.min)
            w = epool.tile([P, 64], F32, tag="w")
            nc.scalar.activation(w[:], mz[:], AF.Exp, bias=0.0, scale=1.0)
            nc.vector.tensor_scalar(w[:], w[:], 1.0, None,
                                    mybir.AluOpType.add)
            r = epool.tile([P, 64], F32, tag="r")
            nc.vector.reciprocal(r[:], w[:])
            nc.scalar.activation(out_ap, r[:], AF.Ln, bias=beps[:], scale=1.0)

        o1 = epool.tile([P, 64], F32)
        logsig(o1[:], sf[:, 64:128], K_SHARP)
        o0 = epool.tile([P, 64], F32)
        logsig(o0[:], sf[:, 0:64], 5.0 * K_SHARP)

        ol = out_logits.ap().rearrange("w (p t) -> w p t", p=P, t=BATCH // P)
        nc.sync.dma_start(ol[0], o1[:])
        nc.sync.dma_start(ol[1], o0[:])


_NC_CACHE = {}


def _get_nc(to_count, ti):
    key = (to_count, ti)
    if key not in _NC_CACHE:
        _NC_CACHE[key] = build_nc(to_count, ti)
    return _NC_CACHE[key]


def make_in_maps(sub_logits, original_indices, to_count, ti):
    idx = np.asarray(original_indices).astype(np.int32)
    v = np.asarray(sub_logits, dtype=np.float32)
    w0 = v[:, 0].astype(np.float16)
    w12 = (v[:, 1] + v[:, 2]).astype(np.float16)
    q_f = (idx & 63).astype(np.float16)
    # duplicated-pair element layout: [w0, w0, w12, w12, q, q]
    packed = np.empty((TOTAL, 6), dtype=np.float16)
    packed[:, 0] = w0
    packed[:, 1] = w0
    packed[:, 2] = w12
    packed[:, 3] = w12
    packed[:, 4] = q_f
    packed[:, 5] = q_f
    packed = packed.reshape(N_CORES, SHARD * 6)

    # r one-hot prebuilt as fp8e4m3 bytes (1.0 = 0x38)
    b3 = np.zeros((TOTAL, 128), dtype=np.uint8)
    b3[np.arange(TOTAL), idx >> 6] = ONE_F8
    b3 = b3.reshape(N_CORES, SHARD * 128)

    # q one-hot prebuilt in fp16 for every 4th block (to % 4 == 2)
    nblk = SHARD // (P * ti)
    q4 = (idx & 63).reshape(N_CORES, nblk, P, ti)[:, 2::4]
    ohq = np.zeros(q4.shape + (64,), dtype=np.float16)
    np.put_along_axis(
        ohq, q4[..., None].astype(np.int64),
        np.ones_like(q4, dtype=np.float16)[..., None], axis=4,
    )
    ohq = ohq.reshape(N_CORES, (SHARD // 4) * 64)

    io64 = np.ascontiguousarray(
        np.broadcast_to(np.arange(64, dtype=np.float16), (P, 64))
    )
    return [
        {"data": packed[c], "b3oh": b3[c], "ohq16": ohq[c], "iota64": io64}
        for c in range(N_CORES)
    ]


def kernel(sub_logits, original_indices, batch_size=None, _trace=False):
    to_count, ti = 64, 128
    nc = _get_nc(to_count, ti)
    in_maps = make_in_maps(sub_logits, original_indices, to_count, ti)
    res = run_bass_kernel_spmd(
        nc, in_maps, core_ids=list(range(N_CORES)), trace=_trace
    )
    logits = res.results[0]["logits"]
    out = np.stack([logits[0], logits[1]], axis=1).astype(np.float32)
    if _trace:
        kernel._last_results = res
    return out
